# revision 16
# baseline (speedup 1.0000x reference)
"""Self-contained Trainium2 Bass kernel for nn_DariushLayer_14087492731059.

kernel(**inputs) takes the FULL unsharded inputs of reference.setup_inputs()
and returns the full [B, S, D] float32 output, computed across 8 NeuronCores.

Sharding: attention tensor-parallel over heads (2 heads/core), MoE
expert-parallel (1 expert/core).  The attention->MoE boundary is a per-batch
bf16 ReduceScatter (each core owns a 256-token chunk per batch), followed by
an owner-side rmsnorm + transpose, then a per-batch fp8 AllGather of the
normalized/transposed activations with an f32 router-logit sidecar.  The MoE
FFN runs in fp8 (e4m3) with DoubleRow matmuls (2x PE rate); router gates are
computed from the f32 logit sidecar.  Host sums the per-core partial outputs
(bf16) plus the owner-chunk residual stream h (f32).
"""

import numpy as np
import concourse.bass as bass
import concourse.tile as tile
from concourse import bacc, mybir
from contextlib import ExitStack

f32, f32r = mybir.dt.float32, mybir.dt.float32r
bf16 = mybir.dt.bfloat16
fp8 = mybir.dt.float8e4
AF = mybir.ActivationFunctionType
OP = mybir.AluOpType
AX = mybir.AxisListType
PM = mybir.MatmulPerfMode

B, S, D, H, DK, E = 2, 2048, 1024, 16, 64, 8
T = B * S
NC = 8
KC = D // 128
CH = S // NC            # 256: tokens per owner chunk per batch
EPS = 1e-6
MASKNEG = -30000.0
SIDE = E * CH * 4       # sidecar bytes (f32 logits [E, CH])
AGB = 128 * KC * CH + SIDE  # per-core AllGather payload bytes (fp8 hnT + logits)


def build_program():
    nc = bacc.Bacc("TRN2", target_bir_lowering=False, debug=False, num_devices=NC)
    dt = nc.dram_tensor
    io = {}
    def inp(nm, shp, ty=f32):
        io[nm] = dt(nm, shp, ty, kind="ExternalInput").ap()
    def outp(nm, shp, ty=f32):
        io[nm] = dt(nm, shp, ty, kind="ExternalOutput").ap()
    inp("x", [T, D])
    inp("xch", [B, CH, D])
    for nm in ("wq", "wk", "wv", "wqs", "wks"):
        inp(nm, [D, 128], bf16)
    inp("wo", [128, D], bf16)
    inp("cosb", [128, S], bf16); inp("sinb", [128, S], bf16)
    inp("masks", [128, 4, 512], bf16)
    inp("ident", [128, 128]); inp("id64", [128, 128])
    inp("identb", [128, 128], bf16); inp("id64b", [128, 128], bf16)
    inp("rw", [D, E], f32r)
    inp("noise", [T, E])
    inp("sel", [128, E])
    inp("w1", [KC, D, 128], fp8); inp("w2", [KC, D, 128], fp8)
    inp("wout", [D, D], fp8)
    inp("b1h", [128, KC]); inp("b2h", [128, KC])
    outp("out", [T, D], bf16)
    outp("h_out", [B, CH, D])

    with tile.TileContext(nc) as tc, ExitStack() as top:
        const = top.enter_context(tc.tile_pool(name="const", bufs=1))
        psum = top.enter_context(tc.tile_pool(name="psum", bufs=8, space="PSUM"))
        dram = top.enter_context(tc.tile_pool(name="dram", bufs=1, space="DRAM"))

        def P(shape=(128, 512)):
            return psum.tile(list(shape), f32, tag="ps", name="ps")

        cst = {}
        for nm, shp, ty in [("ident", [128, 128], f32), ("id64", [128, 128], f32),
                            ("identb", [128, 128], bf16), ("id64b", [128, 128], bf16),
                            ("sel", [128, E], f32), ("b1h", [128, KC], f32),
                            ("b2h", [128, KC], f32)]:
            cst[nm] = const.tile(shp, ty, name=nm)
            nc.sync.dma_start(cst[nm][:], io[nm][:])
        cst["rw"] = const.tile([128, KC, E], f32r, name="rw")
        nc.sync.dma_start(cst["rw"][:], io["rw"].rearrange("(kc p) m -> p kc m", p=128))
        # fp8 FFN weights: loaded up front, long before they're needed.
        w1r = const.tile([128, KC, KC, 128], fp8, name="w1r")
        w2r = const.tile([128, KC, KC, 128], fp8, name="w2r")
        for hcx in range(KC):
            nc.sync.dma_start(w1r[:, :, hcx, :],
                              io["w1"][hcx].rearrange("(kc p) m -> p kc m", p=128))
            nc.sync.dma_start(w2r[:, :, hcx, :],
                              io["w2"][hcx].rearrange("(kc p) m -> p kc m", p=128))
        wout_t = const.tile([128, KC, D], fp8, name="wout_t")
        nc.sync.dma_start(wout_t[:],
                          io["wout"].rearrange("(hc p) d -> p hc d", p=128))
        ones = const.tile([128, 64], bf16, name="ones")
        nc.vector.memset(ones[:], 1.0)
        onesr = const.tile([128, 64], f32r, name="onesr")
        nc.vector.memset(onesr[:].bitcast(f32), 1.0)
        eps_t = const.tile([128, 1], f32, name="eps_t")
        nc.vector.memset(eps_t[:], EPS)
        wgt_all = const.tile([128, 32], f32, name="wgt_all")

        ar_in = dram.tile([T, D], bf16, name="ar_in")
        rs_out = dram.tile([B, CH, D], bf16, name="rs_out")
        ag_in = dram.tile([B, AGB], fp8, name="ag_in")
        ag_outs = [dram.tile([NC, AGB], fp8, name=f"ag_out{b}", addr_space="Shared")
                   for b in range(B)]

        def _copy(eng, dst, src):
            if eng is nc.scalar:
                nc.scalar.copy(dst, src)
            else:
                eng.tensor_copy(dst, src)
        copy_engines = [nc.vector, nc.scalar]

        # --- rmsnorm one [128, D] row-tile; transpose into xT[:, kc, lo].
        #     rsqrt via exp(-0.5*ln(ms)) keeps Act in the ln/exp table set.
        #     Square+accum on Pool (SBUF-only op; gpsimd cannot touch PSUM).
        def norm_transpose(work, xt, xT, lo, xT8=None):
            sq = work.tile([128, D], f32, tag="sq", name="sq")
            ssum = work.tile([128, 1], f32, tag="ssum", name="ssum")
            nc.scalar.activation(sq[:], xt[:], AF.Square, accum_out=ssum[:])
            lnv = work.tile([128, 1], f32, tag="ssum", name="lnv")
            nc.scalar.activation(lnv[:], ssum[:], AF.Ln, bias=eps_t[:], scale=1.0 / D)
            rr = work.tile([128, 1], f32, tag="ssum", name="rr")
            nc.scalar.activation(rr[:], lnv[:], AF.Exp, scale=-0.5)
            if xT8 is None:
                # phase A: bf16 transpose path (1 cyc/row), copies on DVE
                xh = work.tile([128, D], bf16, tag="xh", name="xh")
                nc.vector.tensor_scalar_mul(xh[:], xt[:], rr[:])
                for kc in range(KC):
                    pt = psum.tile([128, 128], bf16, tag="ps", name="ps")
                    nc.tensor.transpose(pt[:], xh[:, kc * 128:(kc + 1) * 128],
                                        cst["identb"][:])
                    if kc % 2 == 0:
                        nc.vector.tensor_copy(xT[:, kc, lo:lo + 128], pt[:])
                    else:
                        nc.scalar.copy(xT[:, kc, lo:lo + 128], pt[:])
            else:
                # owner: f32 transpose, f32r copy (router) + fp8 copy (AG)
                xh = work.tile([128, D], f32, tag="xhf", name="xhf")
                nc.vector.tensor_scalar_mul(xh[:], xt[:], rr[:])
                for kc in range(KC):
                    pt = P((128, 128))
                    nc.tensor.transpose(pt[:], xh[:, kc * 128:(kc + 1) * 128],
                                        cst["ident"][:])
                    nc.vector.tensor_copy(xT[:, kc, lo:lo + 128], pt[:])
                    nc.scalar.copy(xT8[:, kc, lo:lo + 128], pt[:])

        # =================================================================
        # Phase A: attention (per batch), wo-projection, ReduceScatter
        # =================================================================
        with tc.tile_pool(name="qkv", bufs=1) as qkv, \
             tc.tile_pool(name="apool", bufs=1) as apool:
            qT = qkv.tile([128, T], bf16, name="qT")
            kT = qkv.tile([128, T], bf16, name="kT")
            vT = qkv.tile([128, T], bf16, name="vT")
            oT0 = qkv.tile([64, T], bf16, name="oT0")
            oT1 = qkv.tile([64, T], bf16, name="oT1")
            oTs = [oT0, oT1]
            wo0 = qkv.tile([64, D], bf16, name="wo0")
            wo1 = qkv.tile([64, D], bf16, name="wo1")
            nc.sync.dma_start(wo0[:], io["wo"][0:64, :])
            nc.sync.dma_start(wo1[:], io["wo"][64:128, :])
            for nm, shp, ty in [("cosb", [128, S], bf16), ("sinb", [128, S], bf16),
                                ("masks", [128, 4, 512], bf16)]:
                cst[nm] = apool.tile(shp, ty, name=nm)
                nc.sync.dma_start(cst[nm][:], io[nm][:])
            for nm in ("wq", "wk", "wv", "wqs", "wks"):
                cst[nm] = apool.tile([128, KC, 128], bf16, name=nm)
                nc.sync.dma_start(cst[nm][:],
                                  io[nm].rearrange("(kc p) m -> p kc m", p=128))

            with tc.tile_pool(name="xnt", bufs=2) as xnt_pool, \
                 tc.tile_pool(name="work", bufs=2) as work, \
                 tc.tile_pool(name="att", bufs=3) as att, \
                 tc.tile_pool(name="expp", bufs=4) as expp, \
                 tc.tile_pool(name="vsb", bufs=18) as vsbp:
                for b in range(B):
                    # --- QKV projections + rope for this batch ---
                    for sb in range(4):
                        xnT = xnt_pool.tile([128, KC, 512], bf16, tag="xnT",
                                            name="xnT")
                        for q4 in range(4):
                            st = b * 16 + sb * 4 + q4
                            xt = work.tile([128, D], f32, tag="xt", name="xt")
                            nc.sync.dma_start(xt[:], io["x"][st * 128:(st + 1) * 128, :])
                            norm_transpose(work, xt, xnT, q4 * 128)
                        gl = slice(b * S + sb * 512, b * S + (sb + 1) * 512)
                        sl = slice(sb * 512, (sb + 1) * 512)
                        for base, swp, dst in (("wq", "wqs", qT), ("wk", "wks", kT)):
                            pa = P()
                            for kc in range(KC):
                                nc.tensor.matmul(pa[:], cst[base][:, kc, :],
                                                 xnT[:, kc, :],
                                                 start=(kc == 0), stop=(kc == KC - 1))
                            pb = P()
                            for kc in range(KC):
                                nc.tensor.matmul(pb[:], cst[swp][:, kc, :],
                                                 xnT[:, kc, :],
                                                 start=(kc == 0), stop=(kc == KC - 1))
                            t1 = work.tile([128, 512], f32, tag="t1", name="t1")
                            nc.vector.tensor_tensor(t1[:], pa[:], cst["cosb"][:, sl],
                                                    op=OP.mult)
                            t2 = work.tile([128, 512], f32, tag="t2", name="t2")
                            nc.vector.tensor_tensor(t2[:], pb[:], cst["sinb"][:, sl],
                                                    op=OP.mult)
                            nc.vector.tensor_tensor(dst[:, gl], t1[:], t2[:], op=OP.add)
                        pv = P()
                        for kc in range(KC):
                            nc.tensor.matmul(pv[:], cst["wv"][:, kc, :], xnT[:, kc, :],
                                             start=(kc == 0), stop=(kc == KC - 1))
                        nc.scalar.copy(vT[:, gl], pv[:])

                    # --- attention core for this batch ---
                    for h in range(2):
                        hr = slice(h * 64, (h + 1) * 64)
                        idn = cst["identb"] if h == 0 else cst["id64b"]
                        vchunks = []
                        for m in range(16):
                            gk = slice(b * S + m * 128, b * S + (m + 1) * 128)
                            pt = psum.tile([128, 64], bf16, tag="ps", name="ps")
                            nc.tensor.transpose(pt[:], vT[hr, gk], idn[hr, 0:64])
                            vs = vsbp.tile([128, 65], bf16, tag="vs", name="vs")
                            _copy(copy_engines[m % 2], vs[:, 0:64], pt[:])
                            nc.gpsimd.tensor_copy(vs[:, 64:65], ones[:, 0:1])
                            vchunks.append(vs)
                        for jq in range(4):
                            gq = slice(b * S + jq * 512, b * S + (jq + 1) * 512)
                            nch = 4 * jq + 4
                            pos = P((65, 512))
                            for m in range(nch):
                                gk = slice(b * S + m * 128, b * S + (m + 1) * 128)
                                pse = P()
                                nc.tensor.matmul(pse[:], kT[hr, gk], qT[hr, gq],
                                                 start=True, stop=True)
                                if m >= 4 * jq:
                                    nc.vector.tensor_tensor(
                                        pse[:], pse[:],
                                        cst["masks"][:, m - 4 * jq, :], op=OP.add)
                                et = expp.tile([128, 512], bf16, tag="et", name="et")
                                nc.scalar.activation(et[:], pse[:], AF.Exp, scale=0.125)
                                nc.tensor.matmul(pos[:], vchunks[m][:, 0:65], et[:],
                                                 start=(m == 0), stop=(m == nch - 1))
                            rcpr = att.tile([1, 512], f32r, tag="rcpr", name="rcpr")
                            with nc.allow_low_precision(
                                    reason="f32r is bit-identical to f32"):
                                nc.vector.reciprocal(rcpr[:], pos[64:65, :])
                            bc = P((64, 512))
                            nc.tensor.matmul(bc[:], onesr[0:1, 0:64], rcpr[0:1, :],
                                             start=True, stop=True)
                            bcs = att.tile([64, 512], f32, tag="bcs", name="bcs")
                            nc.scalar.copy(bcs[:], bc[:])
                            nc.vector.tensor_tensor(oTs[h][:, gq], pos[0:64, :],
                                                    bcs[:], op=OP.mult)

                    # --- wo-projection partials for this batch -> ar_in ---
                    for st in range(b * 16, (b + 1) * 16):
                        g = slice(st * 128, (st + 1) * 128)
                        for db in range(2):
                            dsl = slice(db * 512, (db + 1) * 512)
                            pp = P()
                            nc.tensor.matmul(pp[:], oT0[:, g], wo0[:, dsl],
                                             start=True, stop=False)
                            nc.tensor.matmul(pp[:], oT1[:, g], wo1[:, dsl],
                                             start=False, stop=True)
                            ab = att.tile([128, 512], bf16, tag="ab", name="ab")
                            nc.scalar.copy(ab[:], pp[:])
                            nc.sync.dma_start(ar_in[g, dsl], ab[:])

                    # --- per-batch ReduceScatter of attention partials ---
                    nc.gpsimd.collective_compute(
                        "ReduceScatter", OP.add, replica_groups=[list(range(NC))],
                        ins=[ar_in[b * S:(b + 1) * S, :]], outs=[rs_out[b]])

        # =================================================================
        # Owner-chunk norm + router logits + AllGather (per batch)
        # =================================================================
        with tc.tile_pool(name="own", bufs=2) as own, \
             tc.tile_pool(name="ownt", bufs=1) as ownt:
            for b in range(B):
                hnT = ownt.tile([128, KC, CH], f32r, tag="hnT", name="hnT")
                hn8 = ownt.tile([128, KC, CH], fp8, tag="hn8", name="hn8")
                for q in range(CH // 128):
                    lo = q * 128
                    rst = own.tile([128, D], bf16, tag="orst", name="orst")
                    nc.sync.dma_start(rst[:], rs_out[b, lo:lo + 128, :])
                    a2 = own.tile([128, D], f32, tag="oa2", name="oa2")
                    # x rows for this core's chunk: host passes xch = x chunk
                    nc.sync.dma_start(a2[:], io["xch"][b, lo:lo + 128, :])
                    xt = own.tile([128, D], f32, tag="oxt", name="oxt")
                    nc.vector.tensor_tensor(xt[:], a2[:], rst[:], op=OP.add)
                    nc.sync.dma_start(io["h_out"][b, lo:lo + 128, :], xt[:])
                    norm_transpose(own, xt, hnT, lo, xT8=hn8)
                # router logits for this chunk (f32r precision)
                plog = P((E, CH))
                for kc in range(KC):
                    nc.tensor.matmul(plog[:], cst["rw"][:, kc, :], hnT[:, kc, :],
                                     start=(kc == 0), stop=(kc == KC - 1))
                lg = own.tile([E, CH], f32, tag="lg", name="lg")
                nc.vector.tensor_copy(lg[:], plog[:])
                nc.sync.dma_start(
                    ag_in[b, 128 * KC * CH:].bitcast(f32), lg[:])
                nc.sync.dma_start(ag_in[b, 0:128 * KC * CH], hn8[:])
                nc.gpsimd.collective_compute(
                    "AllGather", OP.bypass, replica_groups=[list(range(NC))],
                    ins=[ag_in[b]], outs=[ag_outs[b][:]])

        # =================================================================
        # Phase B: routers (batched) + fp8 DoubleRow expert FFN, per batch
        # =================================================================
        with tc.tile_pool(name="moe", bufs=2) as moe, \
             tc.tile_pool(name="workb", bufs=2) as work:
            HB = 128 * KC * CH
            for b in range(B):
                # routers for all 4 sb groups of this batch
                for sb in range(4):
                    lsb = work.tile([E, 512], f32, tag="lsb", name="lsb")
                    for half in range(2):
                        ch = 2 * sb + half
                        nc.sync.dma_start(
                            lsb[:, half * CH:(half + 1) * CH],
                            ag_outs[b][ch, HB:].bitcast(f32))
                    for q4 in range(4):
                        st = b * 16 + sb * 4 + q4
                        g = slice(st * 128, (st + 1) * 128)
                        ptr = P((128, E))
                        nc.tensor.transpose(ptr[:], lsb[:, q4 * 128:(q4 + 1) * 128],
                                            cst["ident"][0:E, 0:E])
                        nt = work.tile([128, E], f32, tag="nt", name="nt")
                        nc.sync.dma_start(nt[:], io["noise"][g, :])
                        zt = work.tile([128, E], f32, tag="zt", name="zt")
                        nc.vector.tensor_tensor(zt[:], ptr[:], nt[:], op=OP.add)
                        ez = work.tile([128, E], f32, tag="ez", name="ez")
                        den = work.tile([128, 1], f32, tag="den", name="den")
                        nc.scalar.activation(ez[:], zt[:], AF.Exp, accum_out=den[:])
                        rd = work.tile([128, 1], f32, tag="den", name="rd")
                        nc.vector.reciprocal(rd[:], den[:])
                        pet = work.tile([128, E], f32, tag="pet", name="pet")
                        nc.gpsimd.tensor_tensor(pet[:], ez[:], cst["sel"][:],
                                                op=OP.mult)
                        peu = work.tile([128, 1], f32, tag="peu", name="peu")
                        nc.vector.reduce_sum(peu[:], pet[:], axis=AX.X)
                        gtt = work.tile([128, E], f32, tag="gtt", name="gtt")
                        nc.vector.tensor_scalar(gtt[:], ez[:], peu[:], None,
                                                op0=OP.is_gt)
                        cnt = work.tile([128, 1], f32, tag="cnt", name="cnt")
                        nc.vector.reduce_sum(cnt[:], gtt[:], axis=AX.X)
                        ind = work.tile([128, 1], f32, tag="cnt", name="ind")
                        nc.vector.tensor_single_scalar(ind[:], cnt[:], 1.5,
                                                       op=OP.is_lt)
                        pe = work.tile([128, 1], f32, tag="pe", name="pe")
                        nc.vector.tensor_scalar_mul(pe[:], peu[:], rd[:])
                        nc.vector.tensor_tensor(wgt_all[:, st:st + 1], pe[:], ind[:],
                                                op=OP.mult)
                # FFN for this batch
                for sb in range(4):
                    hnT8 = moe.tile([128, KC, 512], fp8, tag="hnT8", name="hnT8")
                    for half in range(2):
                        ch = 2 * sb + half
                        nc.sync.dma_start(
                            hnT8[:, :, half * CH:(half + 1) * CH],
                            ag_outs[b][ch, 0:HB].rearrange(
                                "(p kc t) -> p kc t", p=128, kc=KC))
                    ht = moe.tile([128, KC, 512], fp8, tag="ht", name="ht", bufs=1)
                    for hc in range(KC):
                        p1 = P()
                        for k2 in range(KC // 2):
                            nc.tensor.matmul(p1[:], w1r[:, 2 * k2:2 * k2 + 2, hc, :],
                                             hnT8[:, 2 * k2:2 * k2 + 2, :],
                                             start=(k2 == 0), stop=(k2 == KC // 2 - 1),
                                             perf_mode=PM.DoubleRow)
                        p2 = P()
                        for k2 in range(KC // 2):
                            nc.tensor.matmul(p2[:], w2r[:, 2 * k2:2 * k2 + 2, hc, :],
                                             hnT8[:, 2 * k2:2 * k2 + 2, :],
                                             start=(k2 == 0), stop=(k2 == KC // 2 - 1),
                                             perf_mode=PM.DoubleRow)
                        s1 = work.tile([128, 512], f32, tag="s1", name="s1")
                        nc.scalar.activation(s1[:], p1[:], AF.Silu,
                                             bias=cst["b1h"][:, hc:hc + 1], scale=1.0)
                        nc.vector.scalar_tensor_tensor(
                            ht[:, hc, :], p2[:], cst["b2h"][:, hc:hc + 1], s1[:],
                            op0=OP.add, op1=OP.mult)
                    for q4 in range(4):
                        st = b * 16 + sb * 4 + q4
                        g = slice(st * 128, (st + 1) * 128)
                        for db in range(2):
                            peo = P()
                            for h2 in range(KC // 2):
                                nc.tensor.matmul(
                                    peo[:],
                                    ht[:, 2 * h2:2 * h2 + 2, q4 * 128:(q4 + 1) * 128],
                                    wout_t[:, 2 * h2:2 * h2 + 2,
                                           db * 512:(db + 1) * 512],
                                    start=(h2 == 0), stop=(h2 == KC // 2 - 1),
                                    perf_mode=PM.DoubleRow)
                            ob = work.tile([128, 512], bf16, tag="ob", name="ob")
                            if db == 0:
                                nc.scalar.mul(ob[:], peo[:], wgt_all[:, st:st + 1])
                            else:
                                nc.vector.tensor_scalar_mul(ob[:], peo[:],
                                                            wgt_all[:, st:st + 1])
                            nc.sync.dma_start(io["out"][g, db * 512:(db + 1) * 512],
                                              ob[:])

    nc.compile()
    return nc


# =====================================================================
# Host-side input prep / output combine
# =====================================================================
def prep_in_maps(inputs):
    import ml_dtypes
    x = np.asarray(inputs["x"], np.float32).reshape(T, D)
    scale1 = np.asarray(inputs["scale1"], np.float32)
    scale2 = np.asarray(inputs["scale2"], np.float32)
    wq = scale1[:, None] * np.asarray(inputs["wq"], np.float32)
    wk = scale1[:, None] * np.asarray(inputs["wk"], np.float32)
    wv = scale1[:, None] * np.asarray(inputs["wv"], np.float32)
    wo = np.asarray(inputs["wo"], np.float32)
    rw = scale2[:, None] * np.asarray(inputs["router_w"], np.float32)
    # scale2 folds into the AG'd activations already (they're normed with
    # scale2 == 1 here); fold scale2 into w1/w2 like the baseline did.
    w1 = scale2[None, :, None] * np.asarray(inputs["w1"], np.float32)
    w2 = scale2[None, :, None] * np.asarray(inputs["w2"], np.float32)
    wout = np.asarray(inputs["wout"], np.float32)
    b1 = np.asarray(inputs["b1"], np.float32)
    b2 = np.asarray(inputs["b2"], np.float32)

    import jax
    noise = np.asarray(jax.random.gumbel(jax.random.key(42), (B, S, E),
                                         np.float32)) * 0.05
    noise = noise.reshape(T, E).astype(np.float32)

    half = DK // 2
    inv = 1.0 / (10000.0 ** (np.arange(half, dtype=np.float32) / half))
    ang = np.arange(S, dtype=np.float32)[:, None] * inv[None, :]  # [S, 32]
    cos_h = np.cos(ang).T  # [32, S]
    sin_h = np.sin(ang).T
    blk_cos = np.concatenate([cos_h, cos_h], 0)        # [64, S]
    blk_sin = np.concatenate([sin_h, sin_h], 0)
    cosb = np.concatenate([blk_cos, blk_cos], 0).astype(np.float32)  # [128, S]
    sinb = np.concatenate([blk_sin, blk_sin], 0).astype(np.float32)

    masks = np.zeros((128, 4, 512), np.float32)
    kr = np.arange(128)[:, None]
    qc = np.arange(512)[None, :]
    for t in range(4):
        masks[:, t, :] = np.where(kr + 128 * t <= qc, 0.0, MASKNEG)

    ident = np.eye(128, dtype=np.float32)
    id64 = np.zeros((128, 128), np.float32)
    id64[64:128, 0:64] = np.eye(64, dtype=np.float32)
    identb = ident.astype(ml_dtypes.bfloat16)
    id64b = id64.astype(ml_dtypes.bfloat16)

    xr = x.reshape(B, S, D)
    in_maps = []
    for c in range(NC):
        cols = slice(c * 128, (c + 1) * 128)
        wq_c = np.ascontiguousarray(wq[:, cols])
        wk_c = np.ascontiguousarray(wk[:, cols])
        wv_c = np.ascontiguousarray(wv[:, cols])
        def swap(w):
            ws = np.empty_like(w)
            for hh in range(2):
                r = hh * 64
                ws[:, r:r + 32] = -w[:, r + 32:r + 64]
                ws[:, r + 32:r + 64] = w[:, r:r + 32]
            return ws
        w1_c = np.stack([np.ascontiguousarray(w1[c][:, i * 128:(i + 1) * 128])
                         for i in range(KC)], 0).astype(ml_dtypes.float8_e4m3)
        w2_c = np.stack([np.ascontiguousarray(w2[c][:, i * 128:(i + 1) * 128])
                         for i in range(KC)], 0).astype(ml_dtypes.float8_e4m3)
        sel = np.zeros((128, E), np.float32)
        sel[:, c] = 1.0
        xch = np.ascontiguousarray(xr[:, c * CH:(c + 1) * CH, :])  # [B, CH, D]
        in_maps.append({
            "x": x, "wq": wq_c.astype(ml_dtypes.bfloat16),
            "wk": wk_c.astype(ml_dtypes.bfloat16),
            "wv": wv_c.astype(ml_dtypes.bfloat16),
            "wqs": swap(wq_c).astype(ml_dtypes.bfloat16),
            "wks": swap(wk_c).astype(ml_dtypes.bfloat16),
            "wo": np.ascontiguousarray(wo[cols, :]).astype(ml_dtypes.bfloat16),
            "cosb": cosb.astype(ml_dtypes.bfloat16),
            "sinb": sinb.astype(ml_dtypes.bfloat16),
            "masks": masks.astype(ml_dtypes.bfloat16),
            "ident": ident, "id64": id64, "identb": identb, "id64b": id64b,
            "rw": rw, "noise": noise, "sel": sel,
            "w1": w1_c, "w2": w2_c,
            "wout": np.ascontiguousarray(wout[c]).astype(ml_dtypes.float8_e4m3),
            "b1h": np.ascontiguousarray(b1[c].reshape(KC, 128).T),
            "b2h": np.ascontiguousarray(b2[c].reshape(KC, 128).T),
            "xch": xch,
        })
    return in_maps


def combine(results):
    h = np.zeros((B, S, D), np.float64)
    for c in range(NC):
        h[:, c * CH:(c + 1) * CH, :] = results[c]["h_out"].astype(np.float64)
    y = h.reshape(T, D)
    for c in range(NC):
        y = y + results[c]["out"].astype(np.float64)
    return y.astype(np.float32).reshape(B, S, D)


# ---------------------------------------------------------------------
# PJRT runner (axon): persistent jitted executable for the SPMD launch.
# ---------------------------------------------------------------------
import jax
from jax.sharding import Mesh, PartitionSpec
from jax.experimental.shard_map import shard_map
from concourse import bass2jax
import concourse.mybir as mybir_  # noqa


def make_runner(nc, n_cores):
    bass2jax.install_neuronx_cc_hook()
    partition_name = nc.partition_id_tensor.name if nc.partition_id_tensor else None
    in_names, out_names, out_avals, zero_outs = [], [], [], []
    for alloc in nc.m.functions[0].allocations:
        if not isinstance(alloc, mybir.MemoryLocationSet):
            continue
        name = alloc.memorylocations[0].name
        if alloc.kind == "ExternalInput":
            if name != partition_name:
                in_names.append(name)
        elif alloc.kind == "ExternalOutput":
            out_names.append(name)
            shape = tuple(alloc.tensor_shape)
            dtype = mybir.dt.np(alloc.dtype)
            out_avals.append(jax.core.ShapedArray(shape, dtype))
            zero_outs.append(np.zeros(shape, dtype))
    n_params = len(in_names)
    n_outs = len(out_avals)
    all_in_names = list(in_names) + list(out_names)
    if partition_name is not None:
        all_in_names.append(partition_name)

    def _body(*args):
        operands = list(args)
        if partition_name is not None:
            operands.append(bass2jax.partition_id_tensor())
        outs = bass2jax._bass_exec_p.bind(
            *operands,
            out_avals=tuple(out_avals),
            in_names=tuple(all_in_names),
            out_names=tuple(out_names),
            lowering_input_output_aliases=(),
            sim_require_finite=True,
            sim_require_nnan=True,
            nc=nc,
        )
        return tuple(outs)

    devices = jax.devices()[:n_cores]
    mesh = Mesh(np.asarray(devices), ("core",))
    in_specs = (PartitionSpec("core"),) * (n_params + n_outs)
    out_specs = (PartitionSpec("core"),) * n_outs
    donate = tuple(range(n_params, n_params + n_outs))
    sharded = jax.jit(
        shard_map(_body, mesh=mesh, in_specs=in_specs, out_specs=out_specs,
                  check_rep=False),
        donate_argnums=donate, keep_unused=True,
    )

    def run(in_maps):
        per_core = [[np.asarray(m[name]) for name in in_names] for m in in_maps]
        concat_in = [np.concatenate([per_core[c][i] for c in range(n_cores)], axis=0)
                     for i in range(n_params)]
        concat_zeros = [np.zeros((n_cores * z.shape[0], *z.shape[1:]), z.dtype)
                        for z in zero_outs]
        out_arrs = sharded(*concat_in, *concat_zeros)
        out_arrs = [np.asarray(o) for o in out_arrs]
        return [
            {name: out_arrs[i].reshape(n_cores, *out_avals[i].shape)[c]
             for i, name in enumerate(out_names)}
            for c in range(n_cores)
        ]

    return run


_CACHE = {}


def kernel(**inputs):
    if "nc" not in _CACHE:
        _CACHE["nc"] = build_program()
        _CACHE["run"] = make_runner(_CACHE["nc"], NC)
    in_maps = prep_in_maps(inputs)
    results = _CACHE["run"](in_maps)
    return combine(results)
